# revision 1
# baseline (speedup 1.0000x reference)
"""GNN message-passing kernel for Trainium2 (8 NeuronCores, SPMD).

Strategy: edges sorted by destination node; nodes sharded 7500/core (padded
to 15 windows x 512). Per-core segment-sum is done with one-hot selection
matmuls into PSUM (no collectives). The gated node block runs per window in
transposed layout (features on partitions). Matmuls use float32r (tf32-like).
Host does index prep, the tiny MoAct edge embedding, and xd = x @ lora_down.
"""

import numpy as np

N_NODES, N_EDGES = 60000, 240000
WIDTH, NUM_HEAD, DIM_HEAD = 256, 8, 32
WN = 256  # width_norm
EPS = 1e-6
MINMAX = 20.0 ** 0.5
NC = 8
NPC = 7680          # padded nodes per core (15 windows x 512)
NW = 15             # windows per core
WINN = 512          # nodes per window
P = 128


def _softplus(x):
    return np.logaddexp(0.0, x)


def _host_prep(x, deg, edge_idx, edge_attr, node_elec, lora_down, lora_up,
               emb_edge, moa_w, moa_s, elec_lin, emb_deg, lin_pre, gate_lin,
               gate_kernel, value_lin, value_kernel, act_bias, post_kernel):
    f32 = np.float32
    x = np.asarray(x, f32)
    # xcat = [x | x@lora_down | pad] with a zero row at index N_NODES
    xd = (x @ np.asarray(lora_down, f32)).astype(f32)
    xcat = np.zeros((N_NODES + 1, 292), f32)
    xcat[:N_NODES, :256] = x
    xcat[:N_NODES, 256:288] = xd

    ei0 = np.asarray(edge_idx[0], np.int64)
    ei1 = np.asarray(edge_idx[1], np.int64)
    # host edge embedding (tiny): moa + vocab sum
    diff = node_elec[ei0] - node_elec[ei1]                       # [E,2]
    w = _softplus(np.asarray(moa_w, f32))
    w = w / w.sum(-1, keepdims=True)
    s = _softplus(np.asarray(moa_s, f32))
    moa = (np.tanh(diff[..., None] * s[None]) * w[None]).sum(-1)  # [E,2]
    emb = emb_edge[edge_attr].sum(-2) + moa @ np.asarray(elec_lin, f32)  # [E,32]
    emb = np.asarray(emb, f32)

    order = np.argsort(ei1, kind="stable")
    ei0s, ei1s, embs = ei0[order], ei1[order], emb[order]

    # per-core / per-window tiling
    core_of = ei1s // 7500
    win_of = (ei1s - core_of * 7500) // WINN
    counts = np.zeros((NC, NW), np.int64)
    for c in range(NC):
        m = core_of == c
        counts[c] = np.bincount(win_of[m], minlength=NW)
    tw = [int(np.ceil(counts[:, w].max() / P)) for w in range(NW)]
    tw = [max(t, 1) for t in tw]
    T = sum(tw)

    idx = np.full((NC, T, P, 2), N_NODES, np.int32)
    negslot = np.zeros((NC, T, P, 1), f32)
    embt = np.zeros((NC, T, P, 32), f32)
    starts = np.concatenate([[0], np.cumsum(counts, 1).reshape(NC, NW)[:, :-1].flatten()]).reshape
    for c in range(NC):
        m = core_of == c
        e0, e1, em, wv = ei0s[m], ei1s[m], embs[m], win_of[m]
        off = 0
        pos = np.zeros(NW + 1, np.int64)
        pos[1:] = np.cumsum(counts[c])
        ti = 0
        for wdx in range(NW):
            seg = slice(pos[wdx], pos[wdx + 1])
            n = pos[wdx + 1] - pos[wdx]
            base = ti * P
            fe0 = e0[seg]
            fe1 = e1[seg]
            fem = em[seg]
            flat_i = idx[c].reshape(-1, 2)
            flat_s = negslot[c].reshape(-1, 1)
            flat_e = embt[c].reshape(-1, 32)
            flat_i[base:base + n, 0] = fe0
            flat_i[base:base + n, 1] = fe1
            flat_s[base:base + n, 0] = -(fe1 - (c * 7500 + wdx * WINN)).astype(f32)
            flat_e[base:base + n] = fem
            ti += tw[wdx]

    # one-hot deg  [6, NPC] per core
    onehot = np.zeros((NC, 6, NPC), f32)
    for c in range(NC):
        d = np.asarray(deg[c * 7500:(c + 1) * 7500], np.int64)
        onehot[c, d, np.arange(7500)] = 1.0

    # weights, packed for lhsT use
    lp = np.asarray(lin_pre, f32)
    gl = np.asarray(gate_lin, f32)
    vl = np.asarray(value_lin, f32)
    def pack_256(wm):  # [256,256] -> [128, c, t, 128]
        o = np.zeros((P, 2, 2, P), f32)
        for ci in range(2):
            for t in range(2):
                o[:, ci, t, :] = wm[128 * ci:128 * ci + 128, 128 * t:128 * t + 128]
        return o
    linpre_p = pack_256(lp)
    gatelin_p = pack_256(gl)
    valuelin_p = pack_256(vl)
    loraup_p = np.zeros((32, 2, P), f32)
    lu = np.asarray(lora_up, f32)
    loraup_p[:, 0, :] = lu[:, :128]
    loraup_p[:, 1, :] = lu[:, 128:]
    gk = np.asarray(gate_kernel, f32) / np.sqrt(2.0)
    vk = np.asarray(value_kernel, f32)
    gkpad = np.zeros((P, 8, P), f32)
    vkpad = np.zeros((P, 8, P), f32)
    for h in range(8):
        r = 32 * (h % 4)
        gkpad[r:r + 32, h, :] = gk[h]
        vkpad[r:r + 32, h, :] = vk[h]
    postp = np.zeros((P, 8, 32), f32)
    for h in range(8):
        postp[:, h, :] = post_kernel[h]
    # emb_deg bias folded through gate_kernel' per head: [6, 8, 128]
    ed = np.asarray(emb_deg, f32).reshape(6, 8, 32)
    embdeg_g = np.einsum("dhk,hkf->dhf", ed, gk).astype(f32)
    # ssq block-ones and broadcast selectors
    bo16 = np.zeros((P, 4, 16), f32)
    sel = np.zeros((16, 4, P), f32)
    for t4 in range(4):
        for hp in range(4):
            r = 4 * t4 + hp
            bo16[32 * hp:32 * hp + 32, t4, r] = 1.0
            sel[r, t4, 32 * hp:32 * hp + 32] = 1.0
    iota = np.tile(np.arange(WINN, dtype=f32), (P, 1))
    actb = np.asarray(act_bias, f32).reshape(8, P).T.copy()  # [128, 8]

    shared = dict(xcat=xcat, linpre=linpre_p, gatelin=gatelin_p,
                  valuelin=valuelin_p, loraup=loraup_p, gkpad=gkpad,
                  vkpad=vkpad, postp=postp, embdeg=embdeg_g, bo16=bo16,
                  sel=sel, iota=iota, actb=actb)
    per_core = [dict(idx=idx[c], negslot=negslot[c], embt=embt[c],
                     onehot=onehot[c]) for c in range(NC)]
    return shared, per_core, tw, T


def _build(tw, T):
    import concourse.bass as bass
    import concourse.mybir as mybir
    import concourse.tile as tile
    from concourse import bacc

    F32, F32R, I32 = mybir.dt.float32, mybir.dt.float32r, mybir.dt.int32
    AF = mybir.ActivationFunctionType
    nc = bacc.Bacc("TRN2", target_bir_lowering=False, debug=False,
                   num_devices=NC)

    d_xcat = nc.dram_tensor("xcat", [N_NODES + 1, 292], F32, kind="ExternalInput").ap()
    d_idx = nc.dram_tensor("idx", [T, P, 2], I32, kind="ExternalInput").ap()
    d_neg = nc.dram_tensor("negslot", [T, P, 1], F32, kind="ExternalInput").ap()
    d_emb = nc.dram_tensor("embt", [T, P, 32], F32, kind="ExternalInput").ap()
    d_oh = nc.dram_tensor("onehot", [6, NPC], F32R, kind="ExternalInput").ap()
    d_lp = nc.dram_tensor("linpre", [P, 2, 2, P], F32R, kind="ExternalInput").ap()
    d_gl = nc.dram_tensor("gatelin", [P, 2, 2, P], F32R, kind="ExternalInput").ap()
    d_vl = nc.dram_tensor("valuelin", [P, 2, 2, P], F32R, kind="ExternalInput").ap()
    d_lu = nc.dram_tensor("loraup", [32, 2, P], F32R, kind="ExternalInput").ap()
    d_gk = nc.dram_tensor("gkpad", [P, 8, P], F32R, kind="ExternalInput").ap()
    d_vk = nc.dram_tensor("vkpad", [P, 8, P], F32R, kind="ExternalInput").ap()
    d_pp = nc.dram_tensor("postp", [P, 8, 32], F32R, kind="ExternalInput").ap()
    d_ed = nc.dram_tensor("embdeg", [6, 8, P], F32R, kind="ExternalInput").ap()
    d_bo = nc.dram_tensor("bo16", [P, 4, 16], F32R, kind="ExternalInput").ap()
    d_sel = nc.dram_tensor("sel", [16, 4, P], F32R, kind="ExternalInput").ap()
    d_io = nc.dram_tensor("iota", [P, WINN], F32, kind="ExternalInput").ap()
    d_ab = nc.dram_tensor("actb", [P, 8], F32, kind="ExternalInput").ap()
    d_out = nc.dram_tensor("out", [2, P, NPC], F32, kind="ExternalOutput").ap()

    LO = float(1.0 / MINMAX)
    HI = float(MINMAX)

    with tile.TileContext(nc) as tc:
        with tc.tile_pool(name="const", bufs=1) as cst:
            w_lp = cst.tile([P, 2, 2, P], F32R)
            w_gl = cst.tile([P, 2, 2, P], F32R)
            w_vl = cst.tile([P, 2, 2, P], F32R)
            w_lu = cst.tile([32, 2, P], F32R)
            w_gk = cst.tile([P, 8, P], F32R)
            w_vk = cst.tile([P, 8, P], F32R)
            w_pp = cst.tile([P, 8, 32], F32R)
            w_ed = cst.tile([6, 8, P], F32R)
            w_bo = cst.tile([P, 4, 16], F32R)
            w_sel = cst.tile([16, 4, P], F32R)
            w_io = cst.tile([P, WINN], F32)
            w_ab = cst.tile([P, 8], F32)
            for t_, d_ in ((w_lp, d_lp), (w_gl, d_gl), (w_vl, d_vl),
                           (w_lu, d_lu), (w_gk, d_gk), (w_vk, d_vk),
                           (w_pp, d_pp), (w_ed, d_ed), (w_bo, d_bo),
                           (w_sel, d_sel), (w_io, d_io), (w_ab, d_ab)):
                nc.sync.dma_start(t_[:], d_[:])
            aggm = cst.tile([P, 2, NPC], F32R, tag="aggm")
            aggl = cst.tile([32, NPC], F32R, tag="aggl")
            epsb = cst.tile([P, 1], F32, tag="epsb")
            nc.vector.memset(epsb[:], EPS)

            # ---------------- Phase A: edges ----------------
            with tc.tile_pool(name="pa", bufs=3) as pa, \
                 tc.tile_pool(name="pap", bufs=2, space="PSUM") as pap:
                ti = 0
                for wdx in range(NW):
                    ps_m0 = pap.tile([P, WINN], mybir.dt.float32, tag="pm0")
                    ps_m1 = pap.tile([P, WINN], mybir.dt.float32, tag="pm1")
                    ps_lo = pap.tile([32, WINN], mybir.dt.float32, tag="plo")
                    for j in range(tw[wdx]):
                        i = ti + j
                        it = pa.tile([P, 2], I32, tag="it")
                        nc.sync.dma_start(it[:], d_idx[i, :, :])
                        ng = pa.tile([P, 1], F32, tag="ng")
                        nc.sync.dma_start(ng[:], d_neg[i, :, :])
                        em = pa.tile([P, 32], F32, tag="em")
                        nc.sync.dma_start(em[:], d_emb[i, :, :])
                        g0 = pa.tile([P, 292], F32, tag="g0")
                        g1 = pa.tile([P, 292], F32, tag="g1")
                        nc.gpsimd.indirect_dma_start(
                            out=g0[:], out_offset=None, in_=d_xcat[:],
                            in_offset=bass.IndirectOffsetOnAxis(ap=it[:, 0:1], axis=0))
                        nc.gpsimd.indirect_dma_start(
                            out=g1[:], out_offset=None, in_=d_xcat[:],
                            in_offset=bass.IndirectOffsetOnAxis(ap=it[:, 1:2], axis=0))
                        msg = pa.tile([P, 256], F32R, tag="msg")
                        nc.vector.tensor_add(msg[:], g0[:, :256], g1[:, :256])
                        mod = pa.tile([P, 32], F32R, tag="mod")
                        lowt = pa.tile([P, 32], F32, tag="lowt")
                        nc.vector.tensor_add(lowt[:], g0[:, 256:288], g1[:, 256:288])
                        nc.vector.tensor_mul(mod[:], lowt[:], em[:])
                        t1 = pa.tile([P, WINN], F32, tag="t1")
                        nc.scalar.activation(t1[:], w_io[:], AF.Abs,
                                             bias=ng[:, 0:1], scale=1.0)
                        st = pa.tile([P, WINN], F32R, tag="st")
                        nc.scalar.activation(st[:], t1[:], AF.Relu,
                                             bias=1.0, scale=-1.0)
                        first = j == 0
                        last = j == tw[wdx] - 1
                        nc.tensor.matmul(ps_m0[:], msg[:, 0:128], st[:],
                                         start=first, stop=last)
                        nc.tensor.matmul(ps_m1[:], msg[:, 128:256], st[:],
                                         start=first, stop=last)
                        nc.tensor.matmul(ps_lo[:], mod[:], st[:],
                                         start=first, stop=last)
                    ws = wdx * WINN
                    nc.scalar.activation(aggm[:, 0, ws:ws + WINN], ps_m0[:],
                                         AF.Copy, bias=0.0, scale=1.0)
                    nc.scalar.activation(aggm[:, 1, ws:ws + WINN], ps_m1[:],
                                         AF.Copy, bias=0.0, scale=1.0)
                    nc.vector.tensor_copy(aggl[:, ws:ws + WINN], ps_lo[:])
                    ti += tw[wdx]

            # ---------------- Phase B: nodes ----------------
            with tc.tile_pool(name="pb", bufs=2) as pb, \
                 tc.tile_pool(name="pb1", bufs=1) as pb1, \
                 tc.tile_pool(name="pbp", bufs=1, space="PSUM") as pbp, \
                 tc.tile_pool(name="pbp2", bufs=1, space="PSUM") as pbp2:
                for wdx in range(NW):
                    ws = wdx * WINN
                    sl = slice(ws, ws + WINN)
                    agg = pb.tile([P, 2, WINN], F32R, tag="agg")
                    ohw = pb.tile([6, WINN], F32R, tag="ohw")
                    nc.sync.dma_start(ohw[:], d_oh[:, sl])
                    for t in range(2):
                        ps_a = pbp2.tile([P, WINN], mybir.dt.float32, tag="psa")
                        for c in range(2):
                            nc.tensor.matmul(ps_a[:], w_lp[:, c, t, :],
                                             aggm[:, c, sl], start=(c == 0),
                                             stop=False)
                        nc.tensor.matmul(ps_a[:], w_lu[:, t, :], aggl[:, sl],
                                         start=False, stop=True)
                        nc.scalar.activation(agg[:, t, :], ps_a[:], AF.Copy,
                                             bias=0.0, scale=1.0)
                    xxr = pb1.tile([P, 4, WINN], F32R, tag="xxr")
                    sq = pb1.tile([P, 4, WINN], F32R, tag="sq")
                    for t in range(2):
                        pg = pbp2.tile([P, WINN], mybir.dt.float32, tag="pg")
                        pv = pbp2.tile([P, WINN], mybir.dt.float32, tag="pv")
                        for c in range(2):
                            nc.tensor.matmul(pg[:], w_gl[:, c, t, :],
                                             agg[:, c, :], start=(c == 0),
                                             stop=(c == 1))
                            nc.tensor.matmul(pv[:], w_vl[:, c, t, :],
                                             agg[:, c, :], start=(c == 0),
                                             stop=(c == 1))
                        nc.scalar.activation(xxr[:, t, :], pg[:], AF.Copy,
                                             bias=0.0, scale=1.0)
                        nc.scalar.activation(xxr[:, 2 + t, :], pv[:], AF.Copy,
                                             bias=0.0, scale=1.0)
                        nc.vector.tensor_mul(sq[:, t, :], xxr[:, t, :],
                                             xxr[:, t, :])
                        nc.vector.tensor_mul(sq[:, 2 + t, :], xxr[:, 2 + t, :],
                                             xxr[:, 2 + t, :])
                    ps_q = pbp.tile([16, WINN], mybir.dt.float32, tag="psq")
                    for t4 in range(4):
                        nc.tensor.matmul(ps_q[:], w_bo[:, t4, :], sq[:, t4, :],
                                         start=(t4 == 0), stop=(t4 == 3))
                    lnq = pb.tile([16, WINN], F32, tag="lnq")
                    nc.scalar.activation(lnq[:], ps_q[:], AF.Ln,
                                         bias=epsb[0:16, 0:1],
                                         scale=float(1.0 / 32.0))
                    rstd = pb.tile([16, WINN], F32R, tag="rstd")
                    nc.scalar.activation(rstd[:], lnq[:], AF.Exp,
                                         bias=0.0, scale=-0.5)
                    xxn = pb1.tile([P, 4, WINN], F32R, tag="xxn")
                    for t4 in range(4):
                        pbx = pbp2.tile([P, WINN], mybir.dt.float32, tag="pbx")
                        nc.tensor.matmul(pbx[:], w_sel[:, t4, :], rstd[:],
                                         start=True, stop=True)
                        bcs = pb.tile([P, WINN], F32, tag="bcs")
                        nc.scalar.activation(bcs[:], pbx[:], AF.Copy,
                                             bias=0.0, scale=1.0)
                        nc.vector.tensor_mul(xxn[:, t4, :], xxr[:, t4, :],
                                             bcs[:])
                    outw = pb.tile([P, 2, WINN], F32, tag="outw")
                    for h in range(8):
                        pgg = pbp2.tile([P, WINN], mybir.dt.float32, tag="pgg")
                        nc.tensor.matmul(pgg[:], w_gk[:, h, :],
                                         xxn[:, h // 4, :], start=True,
                                         stop=False)
                        nc.tensor.matmul(pgg[:], w_ed[:, h, :], ohw[:],
                                         start=False, stop=True)
                        pvv = pbp2.tile([P, WINN], mybir.dt.float32, tag="pvv")
                        nc.tensor.matmul(pvv[:], w_vk[:, h, :],
                                         xxn[:, 2 + h // 4, :], start=True,
                                         stop=True)
                        et = pb.tile([P, WINN], F32, tag="et")
                        nc.scalar.activation(et[:], pgg[:], AF.Exp,
                                             bias=w_ab[:, h:h + 1], scale=1.0)
                        at = pb.tile([P, WINN], F32, tag="at")
                        nc.scalar.activation(at[:], et[:], AF.Ln,
                                             bias=1.0, scale=1.0)
                        ac = pb.tile([P, WINN], F32, tag="ac")
                        nc.vector.tensor_scalar(out=ac[:], in0=at[:],
                                                scalar1=LO, scalar2=HI,
                                                op0=mybir.AluOpType.max,
                                                op1=mybir.AluOpType.min)
                        prod = pb.tile([P, WINN], F32R, tag="prod")
                        nc.vector.tensor_mul(prod[:], ac[:], pvv[:])
                        po = pbp2.tile([32, WINN], mybir.dt.float32, tag="po")
                        nc.tensor.matmul(po[:], w_pp[:, h, :], prod[:],
                                         start=True, stop=True)
                        r = 32 * (h % 4)
                        if h % 2 == 0:
                            nc.vector.tensor_copy(outw[r:r + 32, h // 4, :], po[:])
                        else:
                            nc.scalar.activation(outw[r:r + 32, h // 4, :],
                                                 po[:], AF.Copy,
                                                 bias=0.0, scale=1.0)
                    for t in range(2):
                        nc.sync.dma_start(d_out[t, :, sl], outw[:, t, :])
    nc.compile()
    return nc


_CACHE = {}


def kernel(**inputs):
    from concourse.bass_utils import run_bass_kernel_spmd

    shared, per_core, tw, T = _host_prep(**{
        k: np.asarray(v) for k, v in inputs.items()})
    key = tuple(tw)
    if key not in _CACHE:
        _CACHE[key] = _build(tw, T)
    nc = _CACHE[key]
    in_maps = []
    for c in range(NC):
        m = dict(shared)
        m.update(per_core[c])
        m = {k: np.ascontiguousarray(v, np.float32) if v.dtype != np.int32
             else np.ascontiguousarray(v) for k, v in m.items()}
        in_maps.append(m)
    res = run_bass_kernel_spmd(nc, in_maps, core_ids=list(range(NC)))
    outs = []
    for c in range(NC):
        o = res.results[c]["out"]            # [2, 128, NPC]
        full = o.transpose(2, 0, 1).reshape(NPC, 256)[:7500]
        outs.append(full)
    return np.concatenate(outs, 0).astype(np.float32)



# revision 6
# speedup vs baseline: 7.8540x; 7.8540x over previous
"""GNN message-passing kernel for Trainium2 (8 NeuronCores, SPMD).

Strategy: edges sorted by destination node; nodes sharded 7500/core (padded
to 15 windows x 512). Per-core segment-sum via one-hot selection matmuls
into PSUM. The gated node block runs per window in transposed layout.

Host uploads are minimized (the axon host->device link is ~50MB/s for
incompressible data): the node feature table is uploaded *sharded* in fp16
and AllGather'd on-device over NeuronLink; weights are likewise uploaded as
1/8 slices and AllGather'd. Per-edge emb is fp16. Output is fp16. The
jax/shard_map callable is built once and cached (the stock
run_bass_kernel_spmd path re-traces it on every call).
"""

import numpy as np

N_NODES, N_EDGES = 60000, 240000
WIDTH, NUM_HEAD, DIM_HEAD = 256, 8, 32
WN = 256  # width_norm
EPS = 1e-6
MINMAX = 20.0 ** 0.5
NC = 8
NPCORE = 7500       # nodes per core
NPC = 7680          # padded nodes per core (15 windows x 512)
NW = 15             # windows per core
WINN = 512          # nodes per window
P = 128


def _softplus(x):
    return np.logaddexp(0.0, x)


def _host_prep(x, deg, edge_idx, edge_attr, node_elec, lora_down, lora_up,
               emb_edge, moa_w, moa_s, elec_lin, emb_deg, lin_pre, gate_lin,
               gate_kernel, value_lin, value_kernel, act_bias, post_kernel):
    f32, f16 = np.float32, np.float16
    x = np.asarray(x, f32)
    xd = x @ np.asarray(lora_down, f32)
    xcat = np.zeros((N_NODES, 292), f16)
    xcat[:, :256] = x
    xcat[:, 256:288] = xd

    ei0 = np.asarray(edge_idx[0], np.int64)
    ei1 = np.asarray(edge_idx[1], np.int64)
    elec = np.asarray(node_elec, f32)
    diff = elec[ei0] - elec[ei1]                                  # [E,2]
    w = _softplus(np.asarray(moa_w, f32))
    w = w / w.sum(-1, keepdims=True)
    s = _softplus(np.asarray(moa_s, f32))
    moa = (np.tanh(diff[..., None] * s[None]) * w[None]).sum(-1)  # [E,2]
    emb = (np.asarray(emb_edge, f32)[np.asarray(edge_attr)].sum(-2)
           + moa @ np.asarray(elec_lin, f32))                     # [E,32]

    order = np.argsort(ei1, kind="stable")
    e0s, e1s, embs = ei0[order], ei1[order], emb[order]
    gid = (e1s // NPCORE) * NW + (e1s % NPCORE) // WINN           # monotone
    counts = np.bincount(gid, minlength=NC * NW).reshape(NC, NW)
    tw = np.maximum((counts.max(0) + P - 1) // P, 1)              # [NW]
    T = int(tw.sum())
    tile_base = np.zeros(NW, np.int64)
    tile_base[1:] = np.cumsum(tw)[:-1]
    starts = np.zeros(NC * NW, np.int64)
    starts[1:] = np.cumsum(counts.ravel())[:-1]
    rank = np.arange(N_EDGES) - starts[gid]
    core, win = gid // NW, gid % NW
    slot = tile_base[win] * P + rank

    # padding slots gather row 0 but select no column (st == 0 everywhere)
    idx = np.zeros((NC, T * P, 2), np.int32)
    neg = np.full((NC, T * P, 1), -1.0e6, f32)
    embt = np.zeros((NC, T * P, 32), f16)
    idx[core, slot, 0] = e0s
    idx[core, slot, 1] = e1s
    neg[core, slot, 0] = -(e1s - (core * NPCORE + win * WINN)).astype(f32)
    embt[core, slot] = embs.astype(f16)

    oh = np.zeros((NC, 6, NPC), f32)
    n = np.arange(N_NODES)
    oh[n // NPCORE, np.asarray(deg, np.int64), n % NPCORE] = 1.0

    # ---- weights, packed for lhsT use, grouped by row size for AllGather ----
    def pack_256(wm):  # [256,256] -> [128, 2, 2, 128] (row-major 512 floats)
        return np.asarray(wm, f32).reshape(2, P, 2, P).transpose(1, 0, 2, 3)

    sel = np.zeros((16, 4, P), f32)
    bo16 = np.zeros((P, 4, 16), f32)
    for t4 in range(4):
        for hp in range(4):
            r = 4 * t4 + hp
            bo16[32 * hp:32 * hp + 32, t4, r] = 1.0
            sel[r, t4, 32 * hp:32 * hp + 32] = 1.0

    # w512: rows of 512 floats: lin_pre | gate_lin | value_lin | sel -> [400,512]
    w512 = np.concatenate([
        pack_256(lin_pre).reshape(P, 512),
        pack_256(gate_lin).reshape(P, 512),
        pack_256(value_lin).reshape(P, 512),
        sel.reshape(16, 512),
    ], 0)
    # w1024: gk_pad | vk_pad | embdeg_g(pad to 8 rows) -> [264, 1024]
    gk = np.asarray(gate_kernel, f32) / np.sqrt(2.0)
    vk = np.asarray(value_kernel, f32)
    w1024 = np.zeros((264, 8, P), f32)
    for h in range(8):
        r = 32 * (h % 4)
        w1024[r:r + 32, h, :] = gk[h]
        w1024[P + r:P + r + 32, h, :] = vk[h]
    ed = np.asarray(emb_deg, f32).reshape(6, 8, 32)
    w1024[256:262] = np.einsum("dhk,hkf->dhf", ed, gk)
    w1024 = w1024.reshape(264, 1024)
    # w256: post_kernel | lora_up -> [160, 256]
    w256 = np.concatenate([
        np.asarray(post_kernel, f32).transpose(1, 0, 2).reshape(P, 256),
        np.asarray(lora_up, f32).reshape(32, 2, P).reshape(32, 256),
    ], 0)
    w64 = bo16.reshape(P, 64)
    iota = np.tile(np.arange(WINN, dtype=f32), (P, 1))             # [128,512]
    actb = np.asarray(act_bias, f32).reshape(8, P).T.copy()        # [128, 8]

    arrays = dict(
        xcg=xcat,                                  # [60000, 292] f16
        w512=np.ascontiguousarray(w512),           # [400, 512]
        w1024=np.ascontiguousarray(w1024),         # [264, 1024]
        w256=np.ascontiguousarray(w256),           # [160, 256]
        w64=np.ascontiguousarray(w64),             # [128, 64]
        iota=np.ascontiguousarray(iota),           # [128, 512]
        actb=np.ascontiguousarray(actb),           # [128, 8]
        idx=idx.reshape(NC * T, P, 2),
        negslot=neg.reshape(NC * T, P, 1),
        embt=embt.reshape(NC * T, P, 32),
        onehot=oh.reshape(NC * 6, NPC),
    )
    return arrays, tuple(int(t) for t in tw), T


def _build(tw, T):
    import concourse.bass as bass
    import concourse.mybir as mybir
    import concourse.tile as tile
    from concourse import bacc

    F32, F32R, F16, I32 = (mybir.dt.float32, mybir.dt.float32r,
                           mybir.dt.float16, mybir.dt.int32)
    AF = mybir.ActivationFunctionType
    nc = bacc.Bacc("TRN2", target_bir_lowering=False, debug=False,
                   num_devices=NC)

    d_xc = nc.dram_tensor("xcg", [NPCORE, 292], F16, kind="ExternalInput").ap()
    d_w512 = nc.dram_tensor("w512", [50, 512], F32R, kind="ExternalInput").ap()
    d_w1024 = nc.dram_tensor("w1024", [33, 1024], F32R, kind="ExternalInput").ap()
    d_w256 = nc.dram_tensor("w256", [20, 256], F32R, kind="ExternalInput").ap()
    d_w64 = nc.dram_tensor("w64", [16, 64], F32R, kind="ExternalInput").ap()
    d_iota = nc.dram_tensor("iota", [16, 512], F32, kind="ExternalInput").ap()
    d_actb = nc.dram_tensor("actb", [16, 8], F32, kind="ExternalInput").ap()
    d_idx = nc.dram_tensor("idx", [T, P, 2], I32, kind="ExternalInput").ap()
    d_neg = nc.dram_tensor("negslot", [T, P, 1], F32, kind="ExternalInput").ap()
    d_emb = nc.dram_tensor("embt", [T, P, 32], F16, kind="ExternalInput").ap()
    d_oh = nc.dram_tensor("onehot", [6, NPC], F32R, kind="ExternalInput").ap()
    d_out = nc.dram_tensor("out", [2, P, NPC], F16, kind="ExternalOutput").ap()

    LO = float(1.0 / MINMAX)
    HI = float(MINMAX)
    RG = [list(range(NC))]
    BP = mybir.AluOpType.bypass

    with tile.TileContext(nc) as tc:
        with tc.tile_pool(name="dram", bufs=1, space="DRAM") as dp:
            b_xc = dp.tile([NPCORE, 292], F16, tag="b_xc")
            b_w512 = dp.tile([50, 512], F32R, tag="b_w512")
            b_w1024 = dp.tile([33, 1024], F32R, tag="b_w1024")
            b_w256 = dp.tile([20, 256], F32R, tag="b_w256")
            b_w64 = dp.tile([16, 64], F32R, tag="b_w64")
            b_iota = dp.tile([16, 512], F32, tag="b_iota")
            b_actb = dp.tile([16, 8], F32, tag="b_actb")
            g_xc = dp.tile([N_NODES, 292], F16, addr_space="Shared",
                           tag="g_xc")
            g_w512 = dp.tile([400, 512], F32R, addr_space="Shared", tag="g_w512")
            g_w1024 = dp.tile([264, 1024], F32R, addr_space="Shared",
                              tag="g_w1024")
            g_w256 = dp.tile([160, 256], F32R, addr_space="Shared", tag="g_w256")
            g_w64 = dp.tile([128, 64], F32R, addr_space="Shared", tag="g_w64")
            g_iota = dp.tile([128, 512], F32, addr_space="Shared", tag="g_iota")
            g_actb = dp.tile([128, 8], F32, addr_space="Shared", tag="g_actb")

            for b_, d_ in ((b_xc, d_xc), (b_w512, d_w512), (b_w1024, d_w1024),
                           (b_w256, d_w256), (b_w64, d_w64), (b_iota, d_iota),
                           (b_actb, d_actb)):
                nc.sync.dma_start(b_[:], d_[:])
            nc.gpsimd.collective_compute(
                "AllGather", BP, replica_groups=RG, ins=[b_xc[:].opt()],
                outs=[g_xc[:].opt()])
            nc.gpsimd.collective_compute(
                "AllGather", BP, replica_groups=RG, ins=[b_iota[:].opt()],
                outs=[g_iota[:].opt()])
            nc.gpsimd.collective_compute(
                "AllGather", BP, replica_groups=RG, ins=[b_w512[:].opt()],
                outs=[g_w512[:].opt()])
            nc.gpsimd.collective_compute(
                "AllGather", BP, replica_groups=RG, ins=[b_w1024[:].opt()],
                outs=[g_w1024[:].opt()])
            nc.gpsimd.collective_compute(
                "AllGather", BP, replica_groups=RG, ins=[b_w256[:].opt()],
                outs=[g_w256[:].opt()])
            nc.gpsimd.collective_compute(
                "AllGather", BP, replica_groups=RG, ins=[b_w64[:].opt()],
                outs=[g_w64[:].opt()])
            nc.gpsimd.collective_compute(
                "AllGather", BP, replica_groups=RG, ins=[b_actb[:].opt()],
                outs=[g_actb[:].opt()])

            with tc.tile_pool(name="const", bufs=1) as cst:
                w_lp = cst.tile([P, 512], F32R, tag="w_lp")
                w_gl = cst.tile([P, 512], F32R, tag="w_gl")
                w_vl = cst.tile([P, 512], F32R, tag="w_vl")
                w_sel = cst.tile([16, 512], F32R, tag="w_sel")
                w_gk = cst.tile([P, 1024], F32R, tag="w_gk")
                w_vk = cst.tile([P, 1024], F32R, tag="w_vk")
                w_ed = cst.tile([6, 1024], F32R, tag="w_ed")
                w_pp = cst.tile([P, 256], F32R, tag="w_pp")
                w_lu = cst.tile([32, 256], F32R, tag="w_lu")
                w_bo = cst.tile([P, 64], F32R, tag="w_bo")
                w_io = cst.tile([P, 512], F32, tag="w_io")
                w_ab = cst.tile([P, 8], F32, tag="w_ab")
                nc.sync.dma_start(w_lp[:], g_w512[0:128, :])
                nc.sync.dma_start(w_gl[:], g_w512[128:256, :])
                nc.sync.dma_start(w_vl[:], g_w512[256:384, :])
                nc.sync.dma_start(w_sel[:], g_w512[384:400, :])
                nc.sync.dma_start(w_gk[:], g_w1024[0:128, :])
                nc.sync.dma_start(w_vk[:], g_w1024[128:256, :])
                nc.sync.dma_start(w_ed[:], g_w1024[256:262, :])
                nc.sync.dma_start(w_pp[:], g_w256[0:128, :])
                nc.sync.dma_start(w_lu[:], g_w256[128:160, :])
                nc.sync.dma_start(w_bo[:], g_w64[:])
                nc.sync.dma_start(w_io[:], g_iota[:])
                nc.sync.dma_start(w_ab[:], g_actb[:])

                aggm = cst.tile([P, 2, NPC], F32R, tag="aggm")
                aggl = cst.tile([32, NPC], F32R, tag="aggl")
                epsb = cst.tile([P, 1], F32, tag="epsb")
                nc.vector.memset(epsb[:], EPS)

                # ---------------- Phase A: edges ----------------
                with tc.tile_pool(name="pa", bufs=3) as pa, \
                     tc.tile_pool(name="pap", bufs=2, space="PSUM") as pap:
                    ti = 0
                    for wdx in range(NW):
                        ps_m0 = pap.tile([P, WINN], mybir.dt.float32, tag="pm0")
                        ps_m1 = pap.tile([P, WINN], mybir.dt.float32, tag="pm1")
                        ps_lo = pap.tile([32, WINN], mybir.dt.float32, tag="plo")
                        for j in range(tw[wdx]):
                            i = ti + j
                            it = pa.tile([P, 2], I32, tag="it")
                            nc.sync.dma_start(it[:], d_idx[i, :, :])
                            ng = pa.tile([P, 1], F32, tag="ng")
                            nc.sync.dma_start(ng[:], d_neg[i, :, :])
                            em = pa.tile([P, 32], F16, tag="em")
                            nc.sync.dma_start(em[:], d_emb[i, :, :])
                            g0 = pa.tile([P, 292], F16, tag="g0")
                            g1 = pa.tile([P, 292], F16, tag="g1")
                            nc.gpsimd.indirect_dma_start(
                                out=g0[:], out_offset=None, in_=g_xc[:],
                                in_offset=bass.IndirectOffsetOnAxis(
                                    ap=it[:, 0:1], axis=0))
                            nc.gpsimd.indirect_dma_start(
                                out=g1[:], out_offset=None, in_=g_xc[:],
                                in_offset=bass.IndirectOffsetOnAxis(
                                    ap=it[:, 1:2], axis=0))
                            msg = pa.tile([P, 256], F32R, tag="msg")
                            nc.vector.tensor_add(msg[:], g0[:, :256],
                                                 g1[:, :256])
                            mod = pa.tile([P, 32], F32R, tag="mod")
                            lowt = pa.tile([P, 32], F32, tag="lowt")
                            nc.vector.tensor_add(lowt[:], g0[:, 256:288],
                                                 g1[:, 256:288])
                            nc.vector.tensor_mul(mod[:], lowt[:], em[:])
                            t1 = pa.tile([P, WINN], F32, tag="t1")
                            nc.scalar.activation(t1[:], w_io[:], AF.Abs,
                                                 bias=ng[:, 0:1], scale=1.0)
                            st = pa.tile([P, WINN], F32R, tag="st")
                            nc.scalar.activation(st[:], t1[:], AF.Relu,
                                                 bias=1.0, scale=-1.0)
                            first = j == 0
                            last = j == tw[wdx] - 1
                            nc.tensor.matmul(ps_m0[:], msg[:, 0:128], st[:],
                                             start=first, stop=last)
                            nc.tensor.matmul(ps_m1[:], msg[:, 128:256], st[:],
                                             start=first, stop=last)
                            nc.tensor.matmul(ps_lo[:], mod[:], st[:],
                                             start=first, stop=last)
                        ws = wdx * WINN
                        nc.scalar.activation(aggm[:, 0, ws:ws + WINN], ps_m0[:],
                                             AF.Copy, bias=0.0, scale=1.0)
                        nc.scalar.activation(aggm[:, 1, ws:ws + WINN], ps_m1[:],
                                             AF.Copy, bias=0.0, scale=1.0)
                        nc.vector.tensor_copy(aggl[:, ws:ws + WINN], ps_lo[:])
                        ti += tw[wdx]

                # ---------------- Phase B: nodes ----------------
                with tc.tile_pool(name="pb", bufs=2) as pb, \
                     tc.tile_pool(name="pb1", bufs=1) as pb1, \
                     tc.tile_pool(name="pbp", bufs=1, space="PSUM") as pbp, \
                     tc.tile_pool(name="pbp2", bufs=1, space="PSUM") as pbp2:
                    for wdx in range(NW):
                        ws = wdx * WINN
                        sl = slice(ws, ws + WINN)
                        agg = pb.tile([P, 2, WINN], F32R, tag="agg")
                        ohw = pb.tile([6, WINN], F32R, tag="ohw")
                        nc.sync.dma_start(ohw[:], d_oh[:, sl])
                        for t in range(2):
                            ps_a = pbp2.tile([P, WINN], mybir.dt.float32,
                                             tag="psa")
                            for c in range(2):
                                nc.tensor.matmul(
                                    ps_a[:],
                                    w_lp[:, (c * 2 + t) * P:(c * 2 + t + 1) * P],
                                    aggm[:, c, sl], start=(c == 0), stop=False)
                            nc.tensor.matmul(ps_a[:],
                                             w_lu[:, t * P:(t + 1) * P],
                                             aggl[:, sl], start=False,
                                             stop=True)
                            nc.scalar.activation(agg[:, t, :], ps_a[:], AF.Copy,
                                                 bias=0.0, scale=1.0)
                        xxr = pb1.tile([P, 4, WINN], F32R, tag="xxr")
                        sq = pb1.tile([P, 4, WINN], F32R, tag="sq")
                        for t in range(2):
                            pg = pbp2.tile([P, WINN], mybir.dt.float32, tag="pg")
                            pv = pbp2.tile([P, WINN], mybir.dt.float32, tag="pv")
                            for c in range(2):
                                nc.tensor.matmul(
                                    pg[:],
                                    w_gl[:, (c * 2 + t) * P:(c * 2 + t + 1) * P],
                                    agg[:, c, :], start=(c == 0), stop=(c == 1))
                                nc.tensor.matmul(
                                    pv[:],
                                    w_vl[:, (c * 2 + t) * P:(c * 2 + t + 1) * P],
                                    agg[:, c, :], start=(c == 0), stop=(c == 1))
                            nc.scalar.activation(xxr[:, t, :], pg[:], AF.Copy,
                                                 bias=0.0, scale=1.0)
                            nc.scalar.activation(xxr[:, 2 + t, :], pv[:],
                                                 AF.Copy, bias=0.0, scale=1.0)
                            nc.vector.tensor_mul(sq[:, t, :], xxr[:, t, :],
                                                 xxr[:, t, :])
                            nc.vector.tensor_mul(sq[:, 2 + t, :],
                                                 xxr[:, 2 + t, :],
                                                 xxr[:, 2 + t, :])
                        ps_q = pbp.tile([16, WINN], mybir.dt.float32, tag="psq")
                        for t4 in range(4):
                            nc.tensor.matmul(ps_q[:],
                                             w_bo[:, t4 * 16:(t4 + 1) * 16],
                                             sq[:, t4, :],
                                             start=(t4 == 0), stop=(t4 == 3))
                        lnq = pb.tile([16, WINN], F32, tag="lnq")
                        nc.scalar.activation(lnq[:], ps_q[:], AF.Ln,
                                             bias=epsb[0:16, 0:1],
                                             scale=float(1.0 / 32.0))
                        rstd = pb.tile([16, WINN], F32R, tag="rstd")
                        nc.scalar.activation(rstd[:], lnq[:], AF.Exp,
                                             bias=0.0, scale=-0.5)
                        xxn = pb1.tile([P, 4, WINN], F32R, tag="xxn")
                        for t4 in range(4):
                            pbx = pbp2.tile([P, WINN], mybir.dt.float32,
                                            tag="pbx")
                            nc.tensor.matmul(pbx[:],
                                             w_sel[:, t4 * P:(t4 + 1) * P],
                                             rstd[:], start=True, stop=True)
                            bcs = pb.tile([P, WINN], F32, tag="bcs")
                            nc.scalar.activation(bcs[:], pbx[:], AF.Copy,
                                                 bias=0.0, scale=1.0)
                            nc.vector.tensor_mul(xxn[:, t4, :], xxr[:, t4, :],
                                                 bcs[:])
                        outw = pb.tile([P, 2, WINN], F16, tag="outw")
                        for h in range(8):
                            pgg = pbp2.tile([P, WINN], mybir.dt.float32,
                                            tag="pgg")
                            nc.tensor.matmul(pgg[:],
                                             w_gk[:, h * P:(h + 1) * P],
                                             xxn[:, h // 4, :], start=True,
                                             stop=False)
                            nc.tensor.matmul(pgg[:],
                                             w_ed[:, h * P:(h + 1) * P],
                                             ohw[:], start=False, stop=True)
                            pvv = pbp2.tile([P, WINN], mybir.dt.float32,
                                            tag="pvv")
                            nc.tensor.matmul(pvv[:],
                                             w_vk[:, h * P:(h + 1) * P],
                                             xxn[:, 2 + h // 4, :], start=True,
                                             stop=True)
                            et = pb.tile([P, WINN], F32, tag="et")
                            nc.scalar.activation(et[:], pgg[:], AF.Exp,
                                                 bias=w_ab[:, h:h + 1],
                                                 scale=1.0)
                            at = pb.tile([P, WINN], F32, tag="at")
                            nc.scalar.activation(at[:], et[:], AF.Ln,
                                                 bias=1.0, scale=1.0)
                            ac = pb.tile([P, WINN], F32, tag="ac")
                            nc.vector.tensor_scalar(out=ac[:], in0=at[:],
                                                    scalar1=LO, scalar2=HI,
                                                    op0=mybir.AluOpType.max,
                                                    op1=mybir.AluOpType.min)
                            prod = pb.tile([P, WINN], F32R, tag="prod")
                            nc.vector.tensor_mul(prod[:], ac[:], pvv[:])
                            po = pbp2.tile([32, WINN], mybir.dt.float32,
                                           tag="po")
                            nc.tensor.matmul(po[:],
                                             w_pp[:, h * 32:(h + 1) * 32],
                                             prod[:], start=True, stop=True)
                            r = 32 * (h % 4)
                            if h % 2 == 0:
                                nc.vector.tensor_copy(outw[r:r + 32, h // 4, :],
                                                      po[:])
                            else:
                                nc.scalar.activation(outw[r:r + 32, h // 4, :],
                                                     po[:], AF.Copy,
                                                     bias=0.0, scale=1.0)
                        for t in range(2):
                            nc.sync.dma_start(d_out[t, :, sl], outw[:, t, :])
    nc.compile()
    return nc


_BUILT = {}


def _get_runner(key, tw, T):
    if key in _BUILT:
        return _BUILT[key]
    import jax
    import jax.numpy as jnp
    from jax.sharding import Mesh, PartitionSpec, NamedSharding
    try:
        from jax.shard_map import shard_map
    except ImportError:
        from jax.experimental.shard_map import shard_map
    from concourse import mybir
    from concourse.bass2jax import (_bass_exec_p, partition_id_tensor,
                                    install_neuronx_cc_hook)
    install_neuronx_cc_hook()
    nc = _build(tw, T)

    partition_name = (nc.partition_id_tensor.name
                      if nc.partition_id_tensor else None)
    in_names, out_names, out_avals = [], [], []
    for alloc in nc.m.functions[0].allocations:
        if not isinstance(alloc, mybir.MemoryLocationSet):
            continue
        name = alloc.memorylocations[0].name
        if alloc.kind == "ExternalInput":
            if name != partition_name:
                in_names.append(name)
        elif alloc.kind == "ExternalOutput":
            out_names.append(name)
            out_avals.append(jax.core.ShapedArray(
                tuple(alloc.tensor_shape), mybir.dt.np(alloc.dtype)))
    n_params = len(in_names)
    in_names_all = list(in_names) + out_names
    if partition_name is not None:
        in_names_all.append(partition_name)
    donate = tuple(range(n_params, n_params + len(out_names)))

    def _body(*args):
        operands = list(args)
        if partition_name is not None:
            operands.append(partition_id_tensor())
        outs = _bass_exec_p.bind(
            *operands,
            out_avals=tuple(out_avals),
            in_names=tuple(in_names_all),
            out_names=tuple(out_names),
            lowering_input_output_aliases=(),
            sim_require_finite=True,
            sim_require_nnan=True,
            nc=nc,
        )
        return tuple(outs)

    devices = jax.devices()[:NC]
    mesh = Mesh(np.asarray(devices), ("core",))
    nin = n_params + len(out_names)
    fn = jax.jit(
        shard_map(_body, mesh=mesh,
                  in_specs=(PartitionSpec("core"),) * nin,
                  out_specs=(PartitionSpec("core"),) * len(out_names),
                  check_rep=False),
        donate_argnums=donate, keep_unused=True)
    zshapes = [(NC * a.shape[0], *a.shape[1:]) for a in out_avals]
    zdtypes = [a.dtype for a in out_avals]
    sh = NamedSharding(mesh, PartitionSpec("core"))

    def _zeros():
        return tuple(jnp.zeros(s, d) for s, d in zip(zshapes, zdtypes))

    zfn = jax.jit(_zeros, out_shardings=sh)
    _BUILT[key] = (fn, zfn, in_names, out_names)
    return _BUILT[key]


def kernel(**inputs):
    arrays, tw, T = _host_prep(**{k: np.asarray(v) for k, v in inputs.items()})
    fn, zfn, in_names, out_names = _get_runner(tw, tw, T)
    args = [arrays[n] for n in in_names]
    outs = fn(*args, *zfn())
    o = np.asarray(outs[out_names.index("out")])   # [NC*2, 128, NPC] f16
    full = (o.reshape(NC, 2, P, NPC).transpose(0, 3, 1, 2)
            .reshape(NC, NPC, 256)[:, :NPCORE]
            .reshape(N_NODES, 256).astype(np.float32))
    return full
